# revision 6
# baseline (speedup 1.0000x reference)
"""Trainium2 Bass kernel for nn_ExpertsChooseBlock (experts-choose MoE block).

Sharding: pure data-parallel over batch B=8 across 8 NeuronCores (one batch
element per core, no collectives).  Per core:
  P1  x -> out (residual init), x^T via PE transposes, router logits (fp32)
  P2  softmax on device; exact top-512 threshold per expert via gpsimd
      kth_largest + one masked-max chase; index/gate compaction via
      sparse_gather; idx staged to DRAM and reloaded replicated.
  P3  attention branch: dma_gather token rows, LayerNorm (token-major),
      PE-transpose to feature-major, qkv/scores/AV/proj matmuls with
      nested-mask truncation, gate-scaled dma_scatter_add into out.
  P4  MLP branch: same dispatch from updated out, W1/W2 matmuls with
      gelu, gate-scaled dma_scatter_add into out.
"""

import numpy as np

import concourse.bass as bass
import concourse.mybir as mybir
import concourse.tile as tile
from concourse import bacc
from concourse.bass_utils import run_bass_kernel_spmd

F32 = mybir.dt.float32
I16 = mybir.dt.int16
U32 = mybir.dt.uint32
AF = mybir.ActivationFunctionType
ALU = mybir.AluOpType
AX = mybir.AxisListType

B, N, D, E, HEADS, HID = 8, 2048, 768, 4, 12, 3072
CAP = 512
DH = 64
EPS = 1e-5
NT = N // 128          # 16 token tiles
KD = D // 128          # 6 feature tiles

DE = [D >> e for e in range(E)]            # [768, 384, 192, 96]
KDE = [(d + 127) // 128 for d in DE]       # [6, 3, 2, 1]
HIDE = [HID >> e for e in range(E)]        # [3072, 1536, 768, 384]
KHE = [h // 128 for h in HIDE]             # [24, 12, 6, 3]
DPAD = [768, 384, 256, 128]                # scatter elem sizes (256B-aligned)
HEADS_E = []
for _e in range(E):
    hs, d = [], 0
    while d < DE[_e]:
        hs.append((d // DH, min(DH, DE[_e] - d)))
        d += DH
    HEADS_E.append(hs)

# kth_largest: k_adj = (omq*(N-1))>>32 must equal 509 so second output is
# desc[510] (511th largest value).
_OMQ = 1069052418
KTH_Q = 1.0 - _OMQ / 4294967296.0
GELU_MODE = "act"  # "act" (HW table) or "manual" (exact jax tanh formula)


PHASE_MARKS = []  # (phase_name, first_instruction_index) — analysis only


def mark(nc, name):
    import re
    nxt = nc.get_next_instruction_name()
    m = re.search(r"(\d+)$", nxt)
    PHASE_MARKS.append((name, int(m.group(1)) if m else 0))


def ts(i, n):
    return slice(i * n, (i + 1) * n)


F32R = mybir.dt.float32r


def mmr(nc, out, lhsT, rhs, **kw):
    """fp32 matmul in float32r mode (full-rate for free dim >= 256)."""
    nc.tensor.matmul(out, lhsT.bitcast(F32R), rhs.bitcast(F32R), **kw)


def trr(nc, out, in_, ident, **kw):
    nc.tensor.transpose(out, in_, ident, **kw)


def emit(nc, tc, dr, ctx):
    x_d, out_d, idxs_d = dr["x_d"], dr["out_d"], dr["idxs_d"]
    pr_d, mk_d, kv_d, gat_d = dr["pr_d"], dr["mk_d"], dr["kv_d"], dr["gat_d"]

    cpool = ctx.enter_context(tc.tile_pool(name="consts", bufs=1))
    ident = cpool.tile([128, 128], F32, tag="ident")
    nc.sync.dma_start(ident[:], dr["ident_d"][:])
    ones_col = cpool.tile([128, 1], F32, tag="ones_col")
    nc.sync.dma_start(ones_col[:], dr["onesc_d"][:])
    ones_row = cpool.tile([1, 512], F32, tag="ones_row")
    nc.sync.dma_start(ones_row[:], dr["onesr_d"][:])
    onesr_r = cpool.tile([1, 140], F32R, tag="onesr_r")
    nc.sync.dma_start(onesr_r[:], dr["onesb_d"][0:1, :])
    iota1w = cpool.tile([16, 128], F32, tag="iota1w")
    nc.sync.dma_start(iota1w[:], dr["iota_d"][:])

    wr_sb = cpool.tile([128, KD, E], F32, tag="wr")
    nc.sync.dma_start(wr_sb[:], bass.AP(dr["wr_d"], 0, [[E, 128], [128 * E, KD], [1, E]]))

    def vec_sb(dram, cols, tg):
        t = cpool.tile([128, cols], F32, tag=tg, name=tg)
        nc.sync.dma_start(t[:], bass.AP(dram, 0, [[1, 128], [128, cols]]))
        return t

    ln1g = vec_sb(dr["ln1g_d"], KD, "ln1g")
    ln1b = vec_sb(dr["ln1b_d"], KD, "ln1b")
    ln2g = vec_sb(dr["ln2g_d"], KD, "ln2g")
    ln2b = vec_sb(dr["ln2b_d"], KD, "ln2b")
    bproj = vec_sb(dr["bproj_d"], KD, "bproj")
    b1sb = vec_sb(dr["b1_d"], HID // 128, "b1sb")
    b2sb = vec_sb(dr["b2_d"], KD, "b2sb")

    # ---------------- P1: out init, x^T, router logits ----------------
    mark(nc, "P1_xT_router")

    logitsT = cpool.tile([E, N], F32, tag="logitsT")

    with (
        tc.tile_pool(name="xt", bufs=3) as xt_pool,
        tc.tile_pool(name="xTc", bufs=2) as xTc_pool,
        tc.tile_pool(name="pst", bufs=1, space="PSUM") as pst_pool,
        tc.tile_pool(name="psr", bufs=2, space="PSUM") as psr_pool,
    ):
        for g in range(4):
            xTc = xTc_pool.tile([128, KD, 512], F32, tag="xTc")
            pss = [pst_pool.tile([128, 512], F32, tag=f"pst{k}", name=f"pst{k}")
                   for k in range(KD)]
            for t in range(4):
                nt = g * 4 + t
                x_t = xt_pool.tile([128, D], F32, tag="x_t")
                nc.sync.dma_start(x_t[:], x_d[ts(nt, 128), :])
                nc.sync.dma_start(out_d[ts(nt, 128), :], x_t[:])
                for k in range(KD):
                    trr(nc, pss[k][:, ts(t, 128)], x_t[:, ts(k, 128)], ident[:])
            for k in range(KD):
                nc.vector.tensor_copy(xTc[:, k, :], pss[k][:])
            lg = psr_pool.tile([E, 512], F32, tag="lg")
            for k in range(KD):
                nc.tensor.matmul(lg[:], wr_sb[:, k, :], xTc[:, k, :],
                                 start=(k == 0), stop=(k == KD - 1))
            nc.vector.tensor_copy(logitsT[:, ts(g, 512)], lg[:])

    # ---------------- P2: softmax + thresholds + compaction ----------------
    mark(nc, "P2_router_topk")

    idx_sb, gates_tm = [], []
    with (
        tc.tile_pool(name="r2", bufs=1) as r2,
        tc.tile_pool(name="psz", bufs=2, space="PSUM") as psz,
        tc.tile_pool(name="psb", bufs=2, space="PSUM") as psb,
    ):
        expT = r2.tile([E, N], F32, tag="expT")
        rz = r2.tile([1, N], F32, tag="rz")
        probsT = r2.tile([E, N], F32, tag="probsT")
        for g in range(4):
            nc.scalar.activation(expT[:, ts(g, 512)], logitsT[:, ts(g, 512)],
                                 AF.Exp)
            z = psz.tile([1, 512], F32, tag="z")
            nc.tensor.matmul(z[:], ones_col[0:E, :], expT[:, ts(g, 512)],
                             start=True, stop=True)
            nc.vector.reciprocal(rz[:, ts(g, 512)], z[:])
            rzb = psb.tile([E, 512], F32, tag="rzb")
            nc.tensor.matmul(rzb[:], ones_row[0:1, 0:E], rz[:, ts(g, 512)],
                             start=True, stop=True)
            nc.vector.tensor_tensor(probsT[:, ts(g, 512)], expT[:, ts(g, 512)],
                                    rzb[:], ALU.mult)
            nc.sync.dma_start(pr_d[:, ts(g, 512)], probsT[:, ts(g, 512)])
        ptm = r2.tile([128, E, 16], F32, tag="ptm")
        nc.sync.dma_start(ptm[:], bass.AP(pr_d, 0, [[16, 128], [N, E], [1, 16]]))
        kth = r2.tile([1, 2 * E], F32, tag="kth")
        for e in range(E):
            nc.gpsimd.kth_largest(kth[:, ts(e, 2)], ptm[:, e, :],
                                  n_per_lane=16, k=510, quantile=KTH_Q)
        nc.sync.dma_start(kv_d[:], kth[:])
        kv = r2.tile([E, 2], F32, tag="kv")
        nc.sync.dma_start(kv[:], bass.AP(kv_d, 0, [[2, 4], [1, 2]]))
        pm = r2.tile([E, N], F32, tag="pm")
        nc.vector.scalar_tensor_tensor(pm[:], probsT[:], kv[:, 1:2], probsT[:],
                                       op0=ALU.is_lt, op1=ALU.mult)
        v2 = r2.tile([E, 1], F32, tag="v2")
        nc.vector.tensor_reduce(v2[:], pm[:], axis=AX.X, op=ALU.max)
        nc.sync.dma_start(mk_d[0:1, 0:E], v2[:].rearrange("p one -> p (one)" if False
                                                          else "p one -> p one").opt())
        v2w = r2.tile([16, E], F32, tag="v2w")
        nc.sync.dma_start(v2w[:], bass.AP(mk_d, 0, [[0, 16], [1, E]]))

        pw_all = r2.tile([16, E, 128], F32, tag="pw_all")
        nc.sync.dma_start(pw_all[:], bass.AP(pr_d, 0, [[1, 16], [N, E], [16, 128]]))
        for e in range(E):
            pw = pw_all[:, e, :]
            mw = r2.tile([16, 128], F32, tag="mw", bufs=2, name="mw")
            nc.vector.tensor_scalar(mw[:], pw.opt(), v2w[:, e:e + 1], None,
                                    op0=ALU.is_ge)
            tidx = r2.tile([16, 128], F32, tag="tidx", bufs=2)
            nc.vector.tensor_tensor(tidx[:], mw[:], iota1w[:], ALU.mult)
            nc.vector.tensor_scalar(tidx[:], tidx[:], 1.0, None, op0=ALU.subtract)
            gw = r2.tile([16, 128], F32, tag="gw", bufs=2)
            nc.vector.tensor_tensor(gw[:], mw[:], pw[:], ALU.mult)
            nc.vector.scalar_tensor_tensor(gw[:], mw[:], 1.0, gw[:],
                                           op0=ALU.subtract, op1=ALU.add)
            idx_c = r2.tile([16, 32], F32, tag="idx_c", bufs=2)
            nf = r2.tile([1, 1], U32, tag="nf", bufs=2)
            nc.gpsimd.sparse_gather(idx_c[:], tidx[:], num_found=nf[:])
            gat_c = r2.tile([16, 32], F32, tag="gat_c", bufs=2)
            nf2 = r2.tile([1, 1], U32, tag="nf2", bufs=2)
            nc.gpsimd.sparse_gather(gat_c[:], gw[:], num_found=nf2[:])
            idx16 = r2.tile([16, 32], I16, tag="idx16", bufs=2)
            nc.vector.tensor_copy(idx16[:], idx_c[:])
            nc.sync.dma_start(idxs_d[e, :, :], idx16[:])
            isb = cpool.tile([128, 32], I16, tag=f"idx_sb{e}", name=f"idx_sb{e}")
            nc.sync.dma_start(isb[:],
                              bass.AP(idxs_d, e * 512, [[0, 8], [32, 16], [1, 32]]))
            idx_sb.append(isb)
            nc.sync.dma_start(gat_d[e:e + 1, :], gat_c[:])
            gtm = cpool.tile([128, 4], F32, tag=f"gates{e}", name=f"gates{e}")
            nc.sync.dma_start(gtm[:],
                              bass.AP(gat_d, e * 512, [[1, 8], [32, 16], [8, 4]]))
            gates_tm.append(gtm)

    # ---------------- shared helpers ----------------
    def ln_tiles(xg, g_sb, b_sb, dstT, e, xp, psp, pstag, on_act):
        kde = KDE[e]
        rs_, nmr_ = [], []
        for t in range(4):
            if (t % 2 == 0) if on_act == 2 else on_act:
                dump = xp.tile([128, D], F32, tag="ln_dump", bufs=2, name="ln_dump")
                acc = xp.tile([128, 2], F32, tag="ln_acc", bufs=4, name="ln_acc")
                nc.scalar.activation(dump[:], xg[:, t, :], AF.Square,
                                     accum_out=acc[:, 0:1])
                nc.scalar.activation(dump[:], xg[:, t, :], AF.Identity,
                                     accum_out=acc[:, 1:2])
                ag = xp.tile([128, 2], F32, tag="ln_ag", bufs=4, name="ln_ag")
                nc.vector.tensor_scalar(ag[:, 0:1], acc[:, 1:2], 1.0 / D, None,
                                        op0=ALU.mult)
                m2 = xp.tile([128, 2], F32, tag="ln_m2", bufs=4, name="ln_m2")
                nc.vector.tensor_tensor(m2[:, 0:1], ag[:, 0:1], ag[:, 0:1],
                                        ALU.mult)
                nc.vector.scalar_tensor_tensor(m2[:, 1:2], acc[:, 0:1], 1.0 / D,
                                               m2[:, 0:1], op0=ALU.mult,
                                               op1=ALU.subtract)
                ve = xp.tile([128, 2], F32, tag="ln_ve", bufs=4, name="ln_ve")
                nc.vector.tensor_scalar(ve[:, 0:1], m2[:, 1:2], EPS, None,
                                        op0=ALU.add)
            else:
                st = xp.tile([128, 12], F32, tag="ln_st", name="ln_st")
                nc.vector.bn_stats(st[:, 0:6], xg[:, t, 0:384])
                nc.vector.bn_stats(st[:, 6:12], xg[:, t, 384:768])
                ag = xp.tile([128, 2], F32, tag="ln_ag", bufs=4, name="ln_ag")
                nc.vector.bn_aggr(ag[:], st[:])
                ve = xp.tile([128, 2], F32, tag="ln_ve", bufs=4, name="ln_ve")
                nc.vector.tensor_scalar(ve[:, 0:1], ag[:, 1:2], EPS, None,
                                        op0=ALU.add)
            nc.vector.reciprocal(ve[:, 1:2], ve[:, 0:1])
            rs_.append(ve)
            nmr_.append(ag)
        r4 = xp.tile([128, 8], F32, tag="ln_r4", name="ln_r4")
        for t in range(4):
            nc.scalar.activation(r4[:, 2 * t:2 * t + 1], rs_[t][:, 1:2], AF.Sqrt)
        for t in range(4):
            nc.vector.scalar_tensor_tensor(r4[:, 2 * t + 1:2 * t + 2],
                                           nmr_[t][:, 0:1], -1.0,
                                           r4[:, 2 * t:2 * t + 1],
                                           op0=ALU.mult, op1=ALU.mult)
        for t in range(4):
            nc.scalar.activation(xg[:, t, :], xg[:, t, :], AF.Identity,
                                 bias=r4[:, 2 * t + 1:2 * t + 2],
                                 scale=r4[:, 2 * t:2 * t + 1])
        for k in range(kde):
            kp = min(128, DE[e] - k * 128)
            ps = psp.tile([128, 512], F32, tag=pstag, name="ps_ln")
            for t in range(4):
                trr(nc, ps[0:kp, ts(t, 128)],
                    xg[:, t, k * 128:k * 128 + kp], ident[:])
            nc.scalar.activation(dstT[0:kp, k, :], ps[0:kp, :], AF.Identity,
                                 bias=b_sb[0:kp, k:k + 1], scale=g_sb[0:kp, k:k + 1])

    def out_transpose_scatter(yT, e, xp, psp, pstag, ytag, ybufs):
        kde = KDE[e]
        dpad = DPAD[e]
        ytok = xp.tile([128, 4, dpad], F32, tag=ytag, bufs=ybufs, name="ytok")
        if dpad > DE[e]:
            nc.vector.memset(ytok[:, :, DE[e]:dpad], 0.0)
        for k in range(kde):
            kp = min(128, DE[e] - k * 128)
            ps = psp.tile([128, 512], F32, tag=pstag, name="ps_ot")
            for t in range(4):
                trr(nc, ps[:, t * 128:t * 128 + kp],
                    yT[0:kp, k, ts(t, 128)], ident[0:kp, 0:kp])
            for t in range(4):
                nc.vector.tensor_scalar(ytok[:, t, k * 128:k * 128 + kp],
                                        ps[:, t * 128:t * 128 + kp],
                                        gates_tm[e][:, t:t + 1], None, op0=ALU.mult)
        nc.gpsimd.dma_scatter_add(out_d[:, 0:dpad], ytok[:], idx_sb[e][:],
                                  CAP, CAP, dpad, elem_step=D)

    # ---------------- P3: attention branch ----------------
    mark(nc, "P3_attn")

    with (
        tc.tile_pool(name="wqkvp", bufs=1) as wqkv_pool,
        tc.tile_pool(name="wprojp", bufs=1) as wproj_pool,
        tc.tile_pool(name="ax", bufs=1) as ax_pool,
        tc.tile_pool(name="aw", bufs=2) as aw_pool,
        tc.tile_pool(name="psA", bufs=2, space="PSUM") as psA,
        tc.tile_pool(name="psS", bufs=3, space="PSUM") as psS,
        tc.tile_pool(name="psV", bufs=2, space="PSUM") as psV,
        tc.tile_pool(name="psR", bufs=1, space="PSUM") as psR,
    ):
        wqkv = wqkv_pool.tile([128, KD, 3 * D], F32R, tag="wqkv")
        for k in range(KD):
            nc.sync.dma_start(wqkv[:, k, :], dr["wqkv_d"][ts(k, 128), :])
        wproj = wproj_pool.tile([128, KD, D], F32R, tag="wproj")
        for k in range(KD):
            nc.sync.dma_start(wproj[:, k, :], dr["wproj_d"][ts(k, 128), :])

        for e in range(E):
            kde, de = KDE[e], DE[e]
            xg = ax_pool.tile([128, 4, D], F32, tag="xg", bufs=1, name="xg")
            nc.gpsimd.dma_gather(xg[:], x_d[:], idx_sb[e][:], CAP, CAP, D)
            xeT = ax_pool.tile([128, KD, 512], F32R, tag="xeT", bufs=2, name="xeT")
            ln_tiles(xg, ln1g, ln1b, xeT, e, aw_pool, psA, "a", True)

            qT = ax_pool.tile([128, KD, 512], F32R, tag="qT", name="qT")
            kT = ax_pool.tile([128, KD, 512], F32R, tag="kT", name="kT")
            v_sb = ax_pool.tile([128, 4, 12 * 65], mybir.dt.bfloat16, tag="v_sb", name="v_sb")
            for h, dh in HEADS_E[e]:
                nc.vector.memset(v_sb[:, :, h * 65 + dh:(h + 1) * 65], 1.0)
            for mk in range(kde):
                mw_ = min(128, de - mk * 128)
                for dst, coff in ((qT, 0), (kT, D)):
                    ps = psA.tile([128, 512], F32, tag="a", name="ps_qk")
                    for k in range(kde):
                        kp = min(128, de - k * 128)
                        mmr(nc, ps[0:mw_, :],
                            wqkv[0:kp, k, coff + mk * 128:coff + mk * 128 + mw_],
                            xeT[0:kp, k, :], start=(k == 0), stop=(k == kde - 1))
                    nc.vector.tensor_copy(dst[0:mw_, mk, :], ps[0:mw_, :])
            for t in range(4):
                for nsp in range((de + 511) // 512):
                    nw = min(512, de - nsp * 512)
                    ps = psV.tile([128, 512], F32, tag="v", name="ps_v")
                    for k in range(kde):
                        kp = min(128, de - k * 128)
                        mmr(nc, ps[:, 0:nw], xeT[0:kp, k, ts(t, 128)],
                            wqkv[0:kp, k, 2 * D + nsp * 512:2 * D + nsp * 512 + nw],
                            start=(k == 0), stop=(k == kde - 1))
                    for h, dh in HEADS_E[e]:
                        lo = h * DH
                        if lo >= nsp * 512 + nw or lo + dh <= nsp * 512:
                            continue
                        nc.vector.tensor_copy(v_sb[:, t, h * 65:h * 65 + dh],
                                              ps[:, lo - nsp * 512:lo - nsp * 512 + dh])

            o_sb = ax_pool.tile([128, KD, 512], F32R, tag="o_sb", name="o_sb")
            e_sb = ax_pool.tile([128, 4, 512], mybir.dt.bfloat16, tag="e_sb",
                                bufs=2, name="e_sb")
            nh = len(HEADS_E[e])
            os_all = ax_pool.tile([65, 12, 512], mybir.dt.bfloat16, tag="os_all",
                                  name="os_all")
            for h, dh in HEADS_E[e]:
                mk, off = (h * DH) // 128, (h * DH) % 128
                for kc in range(4):
                    sps = psS.tile([128, 512], F32, tag="s", name="ps_s")
                    mmr(nc, sps[:], kT[off:off + dh, mk, ts(kc, 128)],
                        qT[off:off + dh, mk, :], start=True, stop=True)
                    nc.scalar.activation(e_sb[:, kc, :], sps[:], AF.Exp,
                                         scale=float(DH ** -0.5))
                oa = psV.tile([128, 512], F32, tag="v", name="ps_oa")
                for kc in range(4):
                    nc.tensor.matmul(oa[0:dh + 1, :],
                                     v_sb[:, kc, h * 65:h * 65 + dh + 1],
                                     e_sb[:, kc, :], start=(kc == 0), stop=(kc == 3))
                nc.vector.tensor_copy(os_all[0:dh + 1, h, :], oa[0:dh + 1, :])
            for h, dh in HEADS_E[e]:
                mk, off = (h * DH) // 128, (h * DH) % 128
                rs = aw_pool.tile([1, 512], F32R, tag="rs", name="rs")
                nc.vector.reciprocal(rs[:], os_all[dh:dh + 1, h, :].opt())
                rb = psR.tile([128, 512], F32, tag="rb", name="ps_rb")
                mmr(nc, rb[0:dh, :], onesr_r[0:1, 0:dh], rs[:],
                    start=True, stop=True)
                if off == 0:
                    nc.vector.tensor_tensor(o_sb[0:dh, mk, :], os_all[0:dh, h, :],
                                            rb[0:dh, :], ALU.mult)
                else:
                    on = aw_pool.tile([64, 512], F32R, tag="on", name="on")
                    nc.vector.tensor_tensor(on[0:dh, :], os_all[0:dh, h, :],
                                            rb[0:dh, :], ALU.mult)
                    nc.sync.dma_start(o_sb[off:off + dh, mk, :], on[0:dh, :])
            yeT = ax_pool.tile([128, KD, 512], F32, tag="xeT", bufs=2, name="yeT")
            for mk in range(kde):
                mw_ = min(128, de - mk * 128)
                ps = psA.tile([128, 512], F32, tag="a", name="ps_pr")
                for k in range(kde):
                    kp = min(128, de - k * 128)
                    mmr(nc, ps[0:mw_, :],
                        wproj[0:kp, k, mk * 128:mk * 128 + mw_],
                        o_sb[0:kp, k, :], start=(k == 0), stop=(k == kde - 1))
                nc.vector.tensor_scalar(yeT[0:mw_, mk, :], ps[0:mw_, :],
                                        bproj[0:mw_, mk:mk + 1], None, op0=ALU.add)
            out_transpose_scatter(yeT, e, ax_pool, psS, "s", "qT", 1)

    # ---------------- P4: MLP branch ----------------
    mark(nc, "P4_mlp")

    with (
        tc.tile_pool(name="w1p", bufs=1) as w1_pool,
        tc.tile_pool(name="w2p", bufs=5) as w2_pool,
        tc.tile_pool(name="mx", bufs=1) as mx_pool,
        tc.tile_pool(name="mw", bufs=2) as mw_pool,
        tc.tile_pool(name="psM", bufs=2, space="PSUM") as psM,
        tc.tile_pool(name="psY", bufs=1, space="PSUM") as psY,
    ):
        xeTs = []
        for e in range(E):
            xg = mx_pool.tile([128, 4, D], F32, tag="xg2", bufs=2, name="xg2")
            nc.gpsimd.dma_gather(xg[:], out_d[:], idx_sb[e][:], CAP, CAP, D)
            xeT = mx_pool.tile([128, KDE[e], 512], F32R, tag=f"xe2T{e}",
                               name=f"xe2T{e}")
            ln_tiles(xg, ln2g, ln2b, xeT, e, mw_pool, psM, "m", False)
            xeTs.append(xeT)
        for e in range(E):
            kde, de, khe = KDE[e], DE[e], KHE[e]
            xeT = xeTs[e]
            yps = [psY.tile([128, 512], F32, tag=f"y{mk}", name=f"ps_y{mk}")
                   for mk in range(kde)]
            for th in range(khe):
                w1t = w1_pool.tile([128, KD, 128], F32R, tag="w1t", bufs=5,
                                   name="w1t")
                nc.sync.dma_start(
                    w1t[:, 0:kde, :],
                    bass.AP(dr["w1_d"], th * 128,
                            [[HID, 128], [128 * HID, kde], [1, 128]]))
                hps = psM.tile([128, 512], F32, tag="m", name="ps_h")
                for k in range(kde):
                    kp = min(128, de - k * 128)
                    mmr(nc, hps[:], w1t[0:kp, k, :],
                        xeT[0:kp, k, :], start=(k == 0), stop=(k == kde - 1))
                h_sb = mw_pool.tile([128, 512], F32R, tag="h_sb", bufs=3, name="h_sb")
                if GELU_MODE == "act":
                    nc.scalar.activation(h_sb[:], hps[:], AF.Gelu_apprx_tanh,
                                         bias=b1sb[:, th:th + 1])
                else:
                    u = mw_pool.tile([128, 512], F32, tag="g_u", name="g_u")
                    nc.scalar.activation(u[:], hps[:], AF.Identity,
                                         bias=b1sb[:, th:th + 1])
                    t1 = mw_pool.tile([128, 512], F32, tag="g_t1", name="g_t1")
                    nc.vector.tensor_tensor(t1[:], u[:], u[:], ALU.mult)
                    nc.vector.tensor_tensor(t1[:], t1[:], u[:], ALU.mult)
                    nc.vector.scalar_tensor_tensor(t1[:], t1[:], 0.044715, u[:],
                                                   op0=ALU.mult, op1=ALU.add)
                    nc.scalar.activation(t1[:], t1[:], AF.Tanh,
                                         scale=0.7978845608028654)
                    nc.vector.scalar_tensor_tensor(t1[:], t1[:], 1.0, u[:],
                                                   op0=ALU.add, op1=ALU.mult)
                    nc.vector.tensor_scalar(h_sb[:], t1[:], 0.5, None, op0=ALU.mult)
                w2t = w2_pool.tile([128, D], F32R, tag="w2t", name="w2t")
                nc.sync.dma_start(w2t[:, 0:de], dr["w2_d"][ts(th, 128), 0:de])
                for mk in range(kde):
                    mw_ = min(128, de - mk * 128)
                    mmr(nc, yps[mk][0:mw_, :],
                        w2t[:, mk * 128:mk * 128 + mw_], h_sb[:],
                        start=(th == 0), stop=(th == khe - 1))
            y2T = mx_pool.tile([128, KD, 512], F32, tag="y2T", name="y2T")
            for mk in range(kde):
                mw_ = min(128, de - mk * 128)
                nc.vector.tensor_scalar(y2T[0:mw_, mk, :], yps[mk][0:mw_, :],
                                        b2sb[0:mw_, mk:mk + 1], None, op0=ALU.add)
            out_transpose_scatter(y2T, e, mx_pool, psM, "m", "xg2", 2)


def build_nc():
    nc = bacc.Bacc("TRN2", target_bir_lowering=False, debug=False)
    dr = {}
    dr["x_d"] = nc.dram_tensor("x", [N, D], F32, kind="ExternalInput")
    dr["wr_d"] = nc.dram_tensor("Wr", [D, E], F32, kind="ExternalInput")
    dr["ln1g_d"] = nc.dram_tensor("ln1_g", [D], F32, kind="ExternalInput")
    dr["ln1b_d"] = nc.dram_tensor("ln1_b", [D], F32, kind="ExternalInput")
    dr["ln2g_d"] = nc.dram_tensor("ln2_g", [D], F32, kind="ExternalInput")
    dr["ln2b_d"] = nc.dram_tensor("ln2_b", [D], F32, kind="ExternalInput")
    dr["wqkv_d"] = nc.dram_tensor("Wqkv", [D, 3 * D], F32R, kind="ExternalInput")
    dr["wproj_d"] = nc.dram_tensor("Wproj", [D, D], F32R, kind="ExternalInput")
    dr["bproj_d"] = nc.dram_tensor("bproj", [D], F32, kind="ExternalInput")
    dr["w1_d"] = nc.dram_tensor("W1", [D, HID], F32R, kind="ExternalInput")
    dr["b1_d"] = nc.dram_tensor("b1", [HID], F32, kind="ExternalInput")
    dr["w2_d"] = nc.dram_tensor("W2", [HID, D], F32R, kind="ExternalInput")
    dr["b2_d"] = nc.dram_tensor("b2", [D], F32, kind="ExternalInput")
    dr["ident_d"] = nc.dram_tensor("c_ident", [128, 128], F32, kind="ExternalInput")
    dr["onesc_d"] = nc.dram_tensor("c_ones_col", [128, 1], F32, kind="ExternalInput")
    dr["onesr_d"] = nc.dram_tensor("c_ones_row", [1, 512], F32, kind="ExternalInput")
    dr["iota_d"] = nc.dram_tensor("c_iota1w", [16, 128], F32, kind="ExternalInput")
    dr["onesb_d"] = nc.dram_tensor("c_onesb", [128, 140], F32R, kind="ExternalInput")
    dr["out_d"] = nc.dram_tensor("out", [N, D], F32, kind="ExternalOutput")
    dr["idxs_d"] = nc.dram_tensor("idx_stage", [E, 16, 32], I16)
    dr["pr_d"] = nc.dram_tensor("pr_stage", [E, N], F32)
    dr["mk_d"] = nc.dram_tensor("mk_stage", [E, N], F32)
    dr["kv_d"] = nc.dram_tensor("kv_stage", [1, 2 * E], F32)
    dr["gat_d"] = nc.dram_tensor("gat_stage", [E, 512], F32)

    from contextlib import ExitStack
    with tile.TileContext(nc) as tc, ExitStack() as ctx, \
            nc.allow_low_precision(reason="fp32r rounding is intentional"):
        emit(nc, tc, dr, ctx)
    nc.compile()
    return nc


def make_consts():
    iota1w = (np.arange(128)[None, :] * 16 + np.arange(16)[:, None] + 1).astype(
        np.float32)
    return {
        "c_ident": np.eye(128, dtype=np.float32),
        "c_ones_col": np.ones((128, 1), np.float32),
        "c_ones_row": np.ones((1, 512), np.float32),
        "c_iota1w": iota1w,
        "c_onesb": np.ones((128, 140), np.float32),
    }


_NC_CACHE = None


def kernel(**inputs):
    global _NC_CACHE
    if _NC_CACHE is None:
        _NC_CACHE = build_nc()
    nc = _NC_CACHE
    consts = make_consts()
    shared = {k: np.ascontiguousarray(np.asarray(inputs[k], np.float32)) for k in
              ["Wr", "ln1_g", "ln1_b", "ln2_g", "ln2_b", "Wqkv", "Wproj",
               "bproj", "W1", "b1", "W2", "b2"]}
    x = np.asarray(inputs["x"], np.float32)
    in_maps = []
    for b in range(B):
        m = {"x": np.ascontiguousarray(x[b])}
        m.update(shared)
        m.update(consts)
        in_maps.append(m)
    res = run_bass_kernel_spmd(nc, in_maps, core_ids=list(range(B)))
    return np.stack([r["out"] for r in res.results], axis=0)

